# revision 19
# baseline (speedup 1.0000x reference)
"""Trainium2 Bass kernel for nn_ASModel (circle-embedding path-distance punish loss).

Math (exactly equivalent to the reference, verified numerically):
  The reference computes, per (b, n):
      tmp[b,n,:] = k*SCR + pos_dist[b,:] - neg_dist[b,n,:]
  where pos/neg dist are 0.5*(low+high) folds of sums over the path dim of
  gathered embedding rows.  The emb[p1] gather-sum cancels between pos_dist
  and neg_dist, leaving
      tmp[b,n,:] = 0.5*(fold(Sneg[b,n]) - fold(S2[b])) + c[b,n]
      c[b,n]     = SCR * (k[b,n]*margin + diff_pos[b] - diff_neg[b,n])
  with Sneg/S2 raw sums of 8 gathered emb rows, fold(x) = x[:512] + x[512:].
  punish = sum_{b,n} || relu(tmp[b,n,:]) ||_2.

  c comes from tiny integer path-intersection counts ([2048,8] scalars) done
  on host; all embedding-table traffic (the memory-bound part: 147456 row
  gathers x 4KB = 576MB) runs on the 8 NeuronCores, data-parallel over batch.

Device layout per core (256 batches):
  partition p = batch within a 128-batch tile; 2 batch-tiles per core.
  For each (bt, n): one indirect DMA gathers 8 rows (neg path) per partition
  -> [128, 8x1024]; one strided vector reduce sums the 8 rows AND folds the
  two 512-halves in a single op (view [128, 512, 16], reduce last axis).
  Same for the p2 path once per bt.  Then per chunk: subtract, Relu(0.5x+c)
  on scalar engine, Square with accum_out (free-dim sum), Sqrt, accumulate.
  Output: [128, 1] per-batch partial sums; host adds 8x128 values.
"""

import math
import sys

import numpy as np

for _p in ("/opt/trn_rl_repo", "/root/.axon_site/_ro/trn_rl_repo"):
    if _p not in sys.path:
        sys.path.append(_p)

from concourse import bacc, bass, mybir, tile
from concourse.bass_utils import run_bass_kernel_spmd

N_CORES = 8
V, H = 200000, 1024
SD = H // 2
B = 2048
NNEG = 8
PLEN = 8
SCR = 2.0 * math.pi
CIRCLE_MARGIN = 1.0

BPC = B // N_CORES          # 256 batches per core
NBT = BPC // 128            # 2 batch-tiles of 128 partitions
N_NEG_COLS = NBT * NNEG * PLEN      # 128 neg index columns
N_IDX_COLS = N_NEG_COLS + NBT * PLEN  # + 16 p2 columns = 144

_CACHE = {}


def _build_nc():
    fp32 = mybir.dt.float32
    nc = bacc.Bacc(dynamic_dma_scratch_size=32768)
    emb = nc.declare_dram_parameter("emb", [V, H], fp32, isOutput=False)
    idx = nc.declare_dram_parameter("idx", [128, N_IDX_COLS], mybir.dt.int32, isOutput=False)
    cbias = nc.declare_dram_parameter("cbias", [128, NBT * NNEG], fp32, isOutput=False)
    out = nc.declare_dram_parameter("out", [128, 1], fp32, isOutput=True)

    with tile.TileContext(nc) as tc:
        with (
            tc.tile_pool(name="const", bufs=1) as cpool,
            tc.tile_pool(name="gather", bufs=4) as gpool,
            tc.tile_pool(name="small", bufs=3) as spool,
        ):
            idx_t = cpool.tile([128, N_IDX_COLS], mybir.dt.int32)
            c_raw = cpool.tile([128, NBT * NNEG], fp32)
            c_t = cpool.tile([128, NBT * NNEG], fp32)
            acc = cpool.tile([128, 1], fp32)
            nc.sync.dma_start(out=idx_t[:], in_=idx[:])
            nc.sync.dma_start(out=c_raw[:], in_=cbias[:])
            # Plain DVE copy so every later DVE op that reads c only has
            # same-engine deps (scalar-ptr ops have a single sync-wait slot).
            nc.vector.tensor_copy(out=c_t[:], in_=c_raw[:])
            nc.vector.memset(acc[:], 0.0)

            for bt in range(NBT):
                # --- p2 gather-sum-fold -> s2f [128, 512] ---
                p2_t = gpool.tile([128, PLEN * H], fp32, tag="negt")
                pcol = N_NEG_COLS + bt * PLEN
                for j in range(PLEN):
                    nc.gpsimd.indirect_dma_start(
                        out=p2_t[:, j * H:(j + 1) * H],
                        out_offset=None,
                        in_=emb[:],
                        in_offset=bass.IndirectOffsetOnAxis(
                            ap=idx_t[:, pcol + j:pcol + j + 1], axis=0
                        ),
                    )
                # unit-stride in-place binary-tree sum of the 8 rows + fold
                for half in (4096, 2048, 1024):
                    nc.vector.tensor_add(
                        out=p2_t[:, :half],
                        in0=p2_t[:, :half],
                        in1=p2_t[:, half:2 * half],
                    )
                s2f = spool.tile([128, SD], fp32, tag="s2f")
                nc.vector.tensor_add(
                    out=s2f[:], in0=p2_t[:, :SD], in1=p2_t[:, SD:2 * SD]
                )

                for n in range(NNEG):
                    neg_t = gpool.tile([128, PLEN * H], fp32, tag="negt")
                    ncol = (bt * NNEG + n) * PLEN
                    for j in range(PLEN):
                        nc.gpsimd.indirect_dma_start(
                            out=neg_t[:, j * H:(j + 1) * H],
                            out_offset=None,
                            in_=emb[:],
                            in_offset=bass.IndirectOffsetOnAxis(
                                ap=idx_t[:, ncol + j:ncol + j + 1], axis=0
                            ),
                        )
                    col = bt * NNEG + n
                    # s2c = s2f - 2*c[:, col]  (host supplies 2c; the 0.5
                    # scale is folded into the final sqrt via scale=0.25)
                    s2c = spool.tile([128, SD], fp32, tag="s2c")
                    nc.vector.tensor_scalar(
                        out=s2c[:],
                        in0=s2f[:],
                        scalar1=c_t[:, col:col + 1],
                        scalar2=None,
                        op0=mybir.AluOpType.subtract,
                    )
                    for half in (4096, 2048, 1024):
                        nc.vector.tensor_add(
                            out=neg_t[:, :half],
                            in0=neg_t[:, :half],
                            in1=neg_t[:, half:2 * half],
                        )
                    d = spool.tile([128, SD], fp32, tag="d")
                    nc.vector.tensor_add(
                        out=d[:], in0=neg_t[:, :SD], in1=neg_t[:, SD:2 * SD]
                    )
                    # q = negfold - (s2f - 2c) = 2*tmp;  norm fixed by sqrt scale
                    q = spool.tile([128, SD], fp32, tag="q")
                    nc.vector.tensor_tensor(
                        out=q[:], in0=d[:], in1=s2c[:], op=mybir.AluOpType.subtract
                    )
                    u = spool.tile([128, SD], fp32, tag="u")
                    nc.scalar.activation(
                        out=u[:],
                        in_=q[:],
                        func=mybir.ActivationFunctionType.Relu,
                    )
                    sq = spool.tile([128, SD], fp32, tag="sq")
                    ss = spool.tile([128, 1], fp32, tag="ss")
                    nc.scalar.activation(
                        out=sq[:],
                        in_=u[:],
                        func=mybir.ActivationFunctionType.Square,
                        accum_out=ss[:],
                    )
                    rt = spool.tile([128, 1], fp32, tag="rt")
                    nc.scalar.activation(
                        out=rt[:],
                        in_=ss[:],
                        func=mybir.ActivationFunctionType.Sqrt,
                        scale=0.25,
                    )
                    nc.vector.tensor_add(out=acc[:], in0=acc[:], in1=rt[:])

            nc.sync.dma_start(out=out[:], in_=acc[:])
    nc.finalize()
    return nc


def _host_prep(node_embedding, pos_path, neg_path):
    """Compute per-pair bias c[b,n] and per-core index/bias arrays."""
    pos = np.asarray(pos_path).astype(np.int64)
    neg = np.asarray(neg_path).astype(np.int64)
    p1, p2 = pos[:, 0], pos[:, 1]

    inter_pos = (p1[:, :, None] == p2[:, None, :]).any(-1).sum(-1)
    diff_pos = np.maximum(PLEN - inter_pos, 1).astype(np.float32)
    inter_neg = (p1[:, None, :, None] == neg[:, :, None, :]).any(-1).sum(-1)
    diff_neg_raw = (PLEN - inter_neg).astype(np.float32)
    k = diff_neg_raw - 1.0
    diff_neg = np.maximum(diff_neg_raw, 1.0)
    # device consumes 2c (the 0.5 tmp scale is folded into the final sqrt)
    c = (2.0 * SCR * (k * CIRCLE_MARGIN + diff_pos[:, None] - diff_neg)).astype(
        np.float32
    )

    in_maps = []
    emb = np.ascontiguousarray(np.asarray(node_embedding, dtype=np.float32))
    for core in range(N_CORES):
        b0 = core * BPC
        idx_arr = np.empty((128, N_IDX_COLS), dtype=np.int32)
        c_arr = np.empty((128, NBT * NNEG), dtype=np.float32)
        for bt in range(NBT):
            bsl = slice(b0 + bt * 128, b0 + (bt + 1) * 128)
            # neg columns: col = bt*64 + n*8 + j  <- neg[b, n, j]
            idx_arr[:, bt * NNEG * PLEN:(bt + 1) * NNEG * PLEN] = (
                neg[bsl].reshape(128, NNEG * PLEN)
            )
            # p2 columns: col = 128 + bt*8 + j  <- pos[b, 1, j]
            idx_arr[:, N_NEG_COLS + bt * PLEN:N_NEG_COLS + (bt + 1) * PLEN] = p2[bsl]
            c_arr[:, bt * NNEG:(bt + 1) * NNEG] = c[bsl]
        in_maps.append({"emb": emb, "idx": idx_arr, "cbias": c_arr})
    return in_maps


def kernel(node_embedding, pos_path, neg_path):
    if "nc" not in _CACHE:
        _CACHE["nc"] = _build_nc()
    nc = _CACHE["nc"]
    in_maps = _host_prep(node_embedding, pos_path, neg_path)
    res = run_bass_kernel_spmd(nc, in_maps, list(range(N_CORES)))
    _CACHE["last_result"] = res
    total = np.float64(0.0)
    for core in range(N_CORES):
        total += np.asarray(res.results[core]["out"], dtype=np.float64).sum()
    return np.array([total], dtype=np.float32)
